# revision 16
# baseline (speedup 1.0000x reference)
"""DIST loss (hard CE + inter/intra Pearson distillation) on 8 Trainium2 cores.

Data-parallel over batch (4096 rows -> 512/core, 4 blocks of 128 partitions).
Per block the core streams z_s/z_t once from HBM as [128, 8000] f32 tiles
(4.1 MB DMAs near line rate; all of a block's dma_starts are issued a full
block ahead so the Sync queue never runs dry), exponentiates on ScalarE
into fp8 (row sums Zs/Zt free via the activation accumulator), then per
2000-col chunk builds product tiles p11=es^2 / p22=et^2 / p12=es*et with
fused row-sum accumulation: DVE scalar_tensor_tensor and ScalarE
Square-with-accum are the only product+accum forms, both 1x rate, so the
split is chosen to keep each engine just under the per-block DMA pace.
Products do not depend on the 1/Z weights, so they run pipelined 4 chunks
ahead of the matmuls — the first block's products fill the otherwise idle
load-only lead-in window, and the drain tail after the last DMA holds only
12 chunks of products (split ScalarE-heavy, since no exps remain).
Per-column weighted sums via TensorE: 5 accumulating matmuls per 512-col
sub-slice with per-stat weight columns (1/Zs etc.) as stationary; the 4
sub-slices land at PSUM partition groups 0/32/64/96 of one [101, 512]
single-bank tile (explicit tile_position col-groups), so one 512-wide
ScalarE copy evacuates a chunk; evacs are deferred one chunk so they never
head-of-line block the ScalarE queue behind unfinished matmuls.  Stores
are issued from GpSimd (SWDGE) to keep them off the Sync load queue.  The
host sums the partial column stats over blocks/cores and finishes the
O(B + C) scalar math (Pearson means, label gather, log) in float64.
"""
import sys
import types
import numpy as np

sys.path.insert(0, "/opt/trn_rl_repo")

B, C = 4096, 32000
N_CORES = 8
R = B // N_CORES          # 512 rows per core
P = 128                   # partitions
NBLK = R // P             # 4 row blocks per core
DTILE = 8000              # DMA/exp tile width
NDT = C // DTILE          # 4 exp tiles per block per tensor
CK = 2000                 # compute chunk width
NCH = C // CK             # 16 chunks per block
CPT = DTILE // CK         # 4 chunks per exp tile
LOOKAHEAD = 4             # chunks the product passes run ahead of matmuls
PROD_BUFS = 3 * LOOKAHEAD + 2
SUB = 512                 # PSUM sub-slice width
NSUB = 4                  # sub-slices per chunk: 3x512 + 464
EPS = 1e-8

_built = None


def _install_ntff_shim():
    # antenv.axon_hooks is absent in this image; register the ctypes NTFF
    # hook so run_bass_kernel_spmd(trace=True) can profile under axon.
    try:
        import antenv
        import trn_agent_boot.trn_boot as tb
        if "antenv.axon_hooks" in sys.modules:
            return
        hook = tb._ntff_profile_via_ctypes("/opt/axon/libaxon_pjrt.so")
        mod = types.ModuleType("antenv.axon_hooks")
        mod.get_axon_ntff_profile_hook = lambda: hook
        mod.set_axon_ntff_profile_hook = lambda h: None
        antenv.axon_hooks = mod
        sys.modules["antenv.axon_hooks"] = mod
    except Exception:
        pass


def _sub_w(s):
    return SUB if s < NSUB - 1 else CK - (NSUB - 1) * SUB


def _build():
    from contextlib import ExitStack
    import concourse.bacc as bacc
    import concourse.tile as tile
    from concourse import mybir

    f32 = mybir.dt.float32
    bf16 = mybir.dt.bfloat16
    fp8 = mybir.dt.float8e4
    fp8e5 = mybir.dt.float8e5
    Exp = mybir.ActivationFunctionType.Exp
    Square = mybir.ActivationFunctionType.Square
    ADD = mybir.AluOpType.add
    MULT = mybir.AluOpType.mult
    AXF = mybir.AxisListType.X

    nc = bacc.Bacc("TRN2", target_bir_lowering=False, debug=False)
    zs_d = nc.dram_tensor("z_s", [R, C], f32, kind="ExternalInput")
    zt_d = nc.dram_tensor("z_t", [R, C], f32, kind="ExternalInput")
    col_d = nc.dram_tensor("colstats", [NBLK, NCH, NSUB, 5, SUB], bf16,
                           kind="ExternalOutput")
    row_d = nc.dram_tensor("rowstats", [R, 8], f32, kind="ExternalOutput")

    with tile.TileContext(nc) as tc, ExitStack() as ctx:
        zin = ctx.enter_context(tc.tile_pool(name="zin", bufs=3))
        esp = ctx.enter_context(tc.tile_pool(name="esp", bufs=NDT + 1))
        etp = ctx.enter_context(tc.tile_pool(name="etp", bufs=NDT + 1))
        prod = ctx.enter_context(tc.tile_pool(name="prod", bufs=PROD_BUFS))
        statp = ctx.enter_context(tc.tile_pool(name="stat", bufs=3))
        small = ctx.enter_context(tc.tile_pool(name="small", bufs=2))
        psump = ctx.enter_context(tc.tile_pool(name="psum", bufs=4, space="PSUM"))

        es_tiles = [None] * NDT
        et_tiles = [None] * NDT
        z_tiles = {}
        state = {}          # per-block small tiles
        prods = {}          # (b, ci) -> (p11, p22, p12)
        pending_evac = []   # [(b, ci, ps_tile)]

        def emit_load(b, d):
            r0 = b * P
            c0 = d * DTILE
            zs = zin.tile([P, DTILE], f32, tag="zin")
            nc.sync.dma_start(zs[:], zs_d[r0:r0 + P, c0:c0 + DTILE])
            zt = zin.tile([P, DTILE], f32, tag="zin")
            nc.sync.dma_start(zt[:], zt_d[r0:r0 + P, c0:c0 + DTILE])
            z_tiles[(b, d)] = (zs, zt)

        def emit_exp_s(b, d):
            if d == 0:
                st = state[b] = {}
                st["zsp"] = small.tile([P, NDT], f32, tag="zsp", name="zsp")
                st["ztp"] = small.tile([P, NDT], f32, tag="ztp", name="ztp")
                st["u11p"] = small.tile([P, NCH], f32, tag="u11p", name="u11p")
                st["u22p"] = small.tile([P, NCH], f32, tag="u22p", name="u22p")
                st["u12p"] = small.tile([P, NCH], f32, tag="u12p", name="u12p")
            st = state[b]
            zs, _ = z_tiles[(b, d)]
            es = esp.tile([P, DTILE], fp8, tag="es")
            nc.scalar.activation(es[:], zs[:], Exp,
                                 accum_out=st["zsp"][:, d:d + 1])
            es_tiles[d] = es

        def emit_exp_t(b, d):
            st = state[b]
            _, zt = z_tiles.pop((b, d))
            et = etp.tile([P, DTILE], fp8, tag="et")
            nc.scalar.activation(et[:], zt[:], Exp,
                                 accum_out=st["ztp"][:, d:d + 1])
            et_tiles[d] = et

        def emit_wprep(b):
            st = state[b]
            rst = small.tile([P, 8], f32, tag="rst")
            nc.vector.tensor_reduce(rst[:, 0:1], st["zsp"][:, 0:NDT],
                                    axis=AXF, op=ADD)
            nc.vector.tensor_reduce(rst[:, 1:2], st["ztp"][:, 0:NDT],
                                    axis=AXF, op=ADD)
            w1 = small.tile([P, 1], f32, tag="w1")
            nc.vector.reciprocal(w1[:], rst[:, 0:1])
            w2 = small.tile([P, 1], f32, tag="w2")
            nc.vector.reciprocal(w2[:], rst[:, 1:2])
            W = []
            for k in range(5):
                Wk = small.tile([P, 5], bf16, tag=f"W{k}")
                nc.vector.memset(Wk[:], 0.0)
                W.append(Wk)
            nc.vector.tensor_copy(W[0][:, 0:1], w1[:])
            nc.vector.tensor_copy(W[1][:, 1:2], w2[:])
            nc.vector.tensor_mul(W[2][:, 2:3], w1[:], w1[:])
            nc.vector.tensor_mul(W[3][:, 3:4], w2[:], w2[:])
            nc.vector.tensor_mul(W[4][:, 4:5], w1[:], w2[:])
            st["W"] = W
            st["rst"] = rst

        def emit_products(b, ci):
            d, q = divmod(ci, CPT)
            off = q * CK
            esv = es_tiles[d][:, off:off + CK]
            etv = et_tiles[d][:, off:off + CK]
            st = state[b]
            # Engine split: ScalarE's exps dominate its budget in the load
            # windows, so DVE carries most product passes there; the
            # exp-free drain tail (block 3, chunks >= LOOKAHEAD) leans on
            # ScalarE instead.
            if b == NBLK - 1 and ci >= LOOKAHEAD:
                p11_scalar = True
                p22_scalar = ci % 3 == 1
            else:
                p11_scalar = ci % 2 == 0
                p22_scalar = ci == 0

            def emit_prod(src0, src1, acc, on_scalar):
                pt = prod.tile([P, CK], fp8e5, tag="prod", name="pt")
                if on_scalar:
                    nc.scalar.activation(pt[:], src0, Square, accum_out=acc)
                else:
                    nc.vector.scalar_tensor_tensor(pt[:], src0, 1.0, src1,
                                                   MULT, MULT, accum_out=acc)
                return pt

            p11 = emit_prod(esv, esv, st["u11p"][:, ci:ci + 1], p11_scalar)
            p22 = emit_prod(etv, etv, st["u22p"][:, ci:ci + 1], p22_scalar)
            p12 = emit_prod(esv, etv, st["u12p"][:, ci:ci + 1], False)
            prods[(b, ci)] = (esv, etv, p11, p22, p12)

        def emit_pending_evac():
            while pending_evac:
                b0, ci0, ps0 = pending_evac.pop(0)
                st0 = statp.tile([P, SUB], bf16, tag="st")
                nc.scalar.copy(st0[0:101, :], ps0[:])
                for s in range(NSUB):
                    p0 = 32 * s
                    nc.gpsimd.dma_start(col_d[b0, ci0, s],
                                        st0[p0:p0 + 5, 0:SUB])

        def emit_mm(b, ci):
            esv, etv, p11, p22, p12 = prods.pop((b, ci))
            rhs = [esv, etv, p11, p22, p12]
            W = state[b]["W"]
            # 4 sub-slices land at PSUM partition groups 0/32/64/96 of one
            # [101, 512] single-bank tile (explicit tile_position col-groups).
            ps = psump.tile([101, SUB], f32, tag="ps")
            for s in range(NSUB):
                w = _sub_w(s)
                p0 = 32 * s
                for k in range(5):
                    nc.tensor.matmul(ps[p0:p0 + 5, 0:w],
                                     W[k][:, 0:5],
                                     rhs[k][:, s * SUB:s * SUB + w],
                                     start=(k == 0), stop=(k == 4),
                                     tile_position=(0, p0))
            emit_pending_evac()
            pending_evac.append((b, ci, ps))

        def emit_rowfin(b):
            r0 = b * P
            st = state[b]
            rst = st["rst"]
            nc.vector.tensor_reduce(rst[:, 2:3], st["u11p"][:, 0:NCH],
                                    axis=AXF, op=ADD)
            nc.vector.tensor_reduce(rst[:, 3:4], st["u22p"][:, 0:NCH],
                                    axis=AXF, op=ADD)
            nc.vector.tensor_reduce(rst[:, 4:5], st["u12p"][:, 0:NCH],
                                    axis=AXF, op=ADD)
            nc.sync.dma_start(row_d[r0:r0 + P, 0:5], rst[:, 0:5])

        # Lead-in: block 0 loads + exps, plus its first LOOKAHEAD chunks of
        # products (they do not need the 1/Z weights).
        for d in range(NDT):
            emit_load(0, d)
        for d in range(NDT):
            emit_exp_s(0, d)
            emit_exp_t(0, d)
        for ci in range(LOOKAHEAD):
            emit_products(0, ci)
        for b in range(NBLK):
            emit_wprep(b)
            if b + 1 < NBLK:
                for d in range(NDT):
                    emit_load(b + 1, d)
            for ci in range(NCH):
                emit_mm(b, ci)
                if ci + LOOKAHEAD < NCH:
                    emit_products(b, ci + LOOKAHEAD)
                elif b + 1 < NBLK:
                    emit_products(b + 1, ci + LOOKAHEAD - NCH)
                if b + 1 < NBLK and ci % CPT == CPT - 2:
                    emit_exp_s(b + 1, ci // CPT)
                elif b + 1 < NBLK and ci % CPT == CPT - 1:
                    emit_exp_t(b + 1, ci // CPT)
            emit_rowfin(b)
        emit_pending_evac()

    nc.compile()
    return nc


def _get_built():
    global _built
    if _built is None:
        _install_ntff_shim()
        _built = _build()
    return _built


def _unpack_col(colstats):
    """colstats [NBLK, NCH, NSUB, 5, SUB] -> [5, C] float64 column stats."""
    acc = colstats.astype(np.float64).sum(axis=0)   # [NCH, NSUB, 5, SUB]
    col = np.zeros((5, C), np.float64)
    for ci in range(NCH):
        for s in range(NSUB):
            w = _sub_w(s)
            c0 = ci * CK + s * SUB
            col[:, c0:c0 + w] += acc[ci, s][:, 0:w]
    return col


def run_sharded(z_s, z_t, trace=False, tmpdir=None):
    """Run the device program; returns (colstats_sum [5, C] f64,
    rowstats [B, 5] f64, BassKernelResults)."""
    from concourse.bass_utils import run_bass_kernel_spmd

    nc = _get_built()
    z_s = np.ascontiguousarray(np.asarray(z_s, dtype=np.float32))
    z_t = np.ascontiguousarray(np.asarray(z_t, dtype=np.float32))
    in_maps = [
        {"z_s": z_s[i * R:(i + 1) * R], "z_t": z_t[i * R:(i + 1) * R]}
        for i in range(N_CORES)
    ]
    res = run_bass_kernel_spmd(nc, in_maps, core_ids=list(range(N_CORES)),
                               trace=trace, tmpdir=tmpdir)
    col = np.zeros((5, C), np.float64)
    rows = []
    for i in range(N_CORES):
        col += _unpack_col(res.results[i]["colstats"])
        rows.append(res.results[i]["rowstats"][:, :5].astype(np.float64))
    return col, np.concatenate(rows, axis=0), res


def kernel(z_s, z_t, labels):
    col, rowstats, _ = run_sharded(z_s, z_t)
    return _finish(np.asarray(z_s), np.asarray(labels), col, rowstats)


def _finish(z_s, labels, col, rowstats):
    Zs, Zt, U11, U22, U12 = rowstats.T
    invC = 1.0 / C
    # inter: Pearson over classes per row (softmax rows have mean 1/C)
    num = U12 / (Zs * Zt) - invC
    vs = U11 / (Zs * Zs) - invC
    vt = U22 / (Zt * Zt) - invC
    corr = num / (np.sqrt(vs) * np.sqrt(vt) + EPS)
    inter = 1.0 - corr.mean()
    # intra: Pearson over samples per column
    S1, S2, S11, S22, S12 = col
    numc = S12 - S1 * S2 / B
    vsc = S11 - S1 * S1 / B
    vtc = S22 - S2 * S2 / B
    corrc = numc / (np.sqrt(vsc) * np.sqrt(vtc) + EPS)
    intra = 1.0 - corrc.mean()
    # hard CE: mean(logsumexp(z_s) - z_s[label])
    lab = np.asarray(labels).astype(np.int64).ravel()
    zl = z_s[np.arange(B), lab].astype(np.float64)
    hard = (np.log(Zs) - zl).mean()
    return np.float32(hard + inter + intra)
